# revision 1
# baseline (speedup 1.0000x reference)
"""IntraAttention Trainium2 kernel, 8-core SPMD.

Reference computation (N=4096 rows, d=1024):
    Q = X @ Wq.T + bq ; K = X @ Wk.T + bk ; V = X @ Wv.T + bv
    alpha = softmax(Q @ K.T / sqrt(d), axis=1)
    V_ = alpha @ V
    x = concat([V_, Q], axis=1)              # [N, 2d]
    x1 = x @ Wl.T + bl                        # [N, d]
    h = x @ Wa.T + ba                         # [N, 2d]
    out = x1 * (h[:, :d] * sigmoid(h[:, d:]))

Sharding: rows of X are sharded across 8 cores (512 rows each). Q stays
local; K and V shards are all-gathered (K as [d, rows] blocks, V as
[rows, d] blocks) in two pipelined chunks each, so each core runs its
512xN attention + GLU chain locally while the gathers fly. The Q-only
halves of the x1/h projections run while the first gather completes.

On-chip everything is computed transposed ([feature, row] layout) so all
matmul contractions run along the partition axis with N=512 moving free
dim. Matmul operands are fp16 (fp22 multiply, fp32 accumulate in PSUM);
biases/normalization/final multiply run in fp32.
"""

import numpy as np

import concourse.bass as bass
import concourse.bacc as bacc
import concourse.tile as tile
import concourse.bass_utils as bass_utils
from concourse import mybir

P = 128            # partitions
D = 1024           # model dim
N = 4096           # rows
NCORES = 8
R = N // NCORES    # rows per core = 512
HR = R // 2        # half of the local rows = 256
DC = D // P        # d chunks = 8
NK = N // P        # key tiles = 32
TD = 2 * D         # 2048
TDC = TD // P      # 16
HK = 4             # k-chunks of the g0 h-projection computed early (Q part)

F32 = mybir.dt.float32
F16 = mybir.dt.float16

RG = [list(range(NCORES))]

# key-tile visit order: (ss-major) so the first 16 tiles only need chunk 0
# of the K/V allgathers. kt_global = rr*4 + ss; softmax/attention are
# permutation-invariant over keys as long as exp tile i pairs with V rows
# of the same key tile.
KT_ORDER = [(rr, ss) for ss in range(4) for rr in range(NCORES)]


def build_nc():
    nc = bacc.Bacc(
        "TRN2",
        target_bir_lowering=False,
        debug=False,
        num_devices=NCORES,
    )

    # ---- per-core I/O ----
    xt = nc.dram_tensor("xt", [D, R], F16, kind="ExternalInput")      # X_c.T
    wqt = nc.dram_tensor("wqt", [D, D], F16, kind="ExternalInput")    # Wq.T
    wkt = nc.dram_tensor("wkt", [D, D], F16, kind="ExternalInput")    # Wk.T
    wvt = nc.dram_tensor("wvt", [D, D], F16, kind="ExternalInput")    # Wv.T
    wlt = nc.dram_tensor("wlt", [TD, D], F16, kind="ExternalInput")   # Wl.T
    wat = nc.dram_tensor("wat", [TD, TD], F16, kind="ExternalInput")  # Wa.T
    bq = nc.dram_tensor("bq", [P, DC], F32, kind="ExternalInput")
    bk = nc.dram_tensor("bk", [P, DC], F32, kind="ExternalInput")
    bvb = nc.dram_tensor("bvb", [P, D], F32, kind="ExternalInput")    # bv bcast
    bl = nc.dram_tensor("bl", [P, DC], F32, kind="ExternalInput")
    ba = nc.dram_tensor("ba", [P, TDC], F32, kind="ExternalInput")
    out = nc.dram_tensor("out", [D, R], F32, kind="ExternalOutput")   # out_c.T

    # ---- collective buffers ----
    ktc_d = [nc.dram_tensor(f"ktc_d{h}", [D, HR], F16) for h in range(2)]
    vc_d = [nc.dram_tensor(f"vc_d{h}", [HR, D], F16) for h in range(2)]
    ag_k = [nc.dram_tensor(f"ag_k{h}", [NCORES * D, HR], F16, addr_space="Shared")
            for h in range(2)]
    ag_v = [nc.dram_tensor(f"ag_v{h}", [NCORES * HR, D], F16, addr_space="Shared")
            for h in range(2)]

    with tile.TileContext(nc) as tc:
        with (
            tc.tile_pool(name="cpool", bufs=1) as cpool,
            tc.tile_pool(name="wpool", bufs=10) as wpool,
            tc.tile_pool(name="vlpool", bufs=4) as vlpool,
            tc.tile_pool(name="pspool", bufs=8, space="PSUM") as pspool,
        ):
            # constants (scalar-engine DMA queue; sync queue kept for bulk)
            bq_t = cpool.tile([P, DC], F32, name="bq_t")
            bk_t = cpool.tile([P, DC], F32, name="bk_t")
            bl_t = cpool.tile([P, DC], F32, name="bl_t")
            ba_t = cpool.tile([P, TDC], F32, name="ba_t")
            bvb_t = cpool.tile([P, D], F32, name="bvb_t")
            nc.scalar.dma_start(bq_t, bq[:, :])
            nc.scalar.dma_start(bk_t, bk[:, :])
            nc.scalar.dma_start(bl_t, bl[:, :])
            nc.scalar.dma_start(ba_t, ba[:, :])
            nc.scalar.dma_start(bvb_t, bvb[:, :])
            ones_t = cpool.tile([P, 1], F16, name="ones_t")
            nc.vector.memset(ones_t, 1.0)
            ones_row = cpool.tile([1, P], F32, name="ones_row")
            nc.vector.memset(ones_row, 1.0)

            with tc.tile_pool(name="qpool", bufs=1) as qpool, \
                 tc.tile_pool(name="vtpool", bufs=1) as vtpool, \
                 tc.tile_pool(name="qfpool", bufs=1) as qfpool:

                # ============ QKV projections + allgathers ============
                with tc.tile_pool(name="xpool", bufs=1) as xpool, \
                     tc.tile_pool(name="stpool", bufs=4) as stpool:
                    xt_t = [xpool.tile([P, R], F16, name=f"xt{k}") for k in range(DC)]

                    # --- K_c.T = Wk @ X_c.T + bk ---  (first: feeds AG(K))
                    kt_ps = [pspool.tile([P, R], F32, name=f"ktps{m}", tag="ps")
                             for m in range(DC)]
                    for k in range(DC):
                        nc.sync.dma_start(xt_t[k], xt[k * P:(k + 1) * P, :])
                        wk_t = wpool.tile([P, D], F16, name="wk_t", tag="w")
                        nc.sync.dma_start(wk_t, wkt[k * P:(k + 1) * P, :])
                        for m in range(DC):
                            nc.tensor.matmul(
                                kt_ps[m], wk_t[:, m * P:(m + 1) * P], xt_t[k],
                                start=(k == 0), stop=(k == DC - 1))
                    for m in range(DC):
                        st = stpool.tile([P, R], F16, name="st_k", tag="st")
                        nc.vector.tensor_scalar_add(st, kt_ps[m], bk_t[:, m:m + 1])
                        for h in range(2):
                            nc.scalar.dma_start(
                                ktc_d[h][m * P:(m + 1) * P, :],
                                st[:, h * HR:(h + 1) * HR])
                    for h in range(2):
                        nc.gpsimd.collective_compute(
                            "AllGather", mybir.AluOpType.bypass, replica_groups=RG,
                            ins=[ktc_d[h].ap().opt()], outs=[ag_k[h].ap().opt()])

                    # --- V_c = X_c @ Wv.T + bv ---
                    v_ps = [pspool.tile([P, R], F32, name=f"vps{i}", tag="ps")
                            for i in range(8)]
                    for k in range(DC):
                        wv_t = wpool.tile([P, D], F16, name="wv_t", tag="w")
                        nc.sync.dma_start(wv_t, wvt[k * P:(k + 1) * P, :])
                        for rt in range(4):
                            for db in range(2):
                                nc.tensor.matmul(
                                    v_ps[rt * 2 + db],
                                    xt_t[k][:, rt * P:(rt + 1) * P],
                                    wv_t[:, db * 512:(db + 1) * 512],
                                    start=(k == 0), stop=(k == DC - 1))
                    for rt in range(4):
                        for db in range(2):
                            st = stpool.tile([P, R], F16, name="st_v", tag="st")
                            nc.vector.tensor_add(
                                st, v_ps[rt * 2 + db], bvb_t[:, db * 512:(db + 1) * 512])
                            nc.scalar.dma_start(
                                vc_d[rt // 2][(rt % 2) * P:(rt % 2 + 1) * P,
                                              db * 512:(db + 1) * 512], st)
                    for h in range(2):
                        nc.gpsimd.collective_compute(
                            "AllGather", mybir.AluOpType.bypass, replica_groups=RG,
                            ins=[vc_d[h].ap().opt()], outs=[ag_v[h].ap().opt()])

                    # --- Q_c.T = Wq @ X_c.T + bq ---
                    qt_t = [qpool.tile([P, R], F16, name=f"qt{m}") for m in range(DC)]
                    q_ps = [pspool.tile([P, R], F32, name=f"qps{m}", tag="ps")
                            for m in range(DC)]
                    for k in range(DC):
                        wq_t = wpool.tile([P, D], F16, name="wq_t", tag="w")
                        nc.sync.dma_start(wq_t, wqt[k * P:(k + 1) * P, :])
                        for m in range(DC):
                            nc.tensor.matmul(
                                q_ps[m], wq_t[:, m * P:(m + 1) * P], xt_t[k],
                                start=(k == 0), stop=(k == DC - 1))
                    for m in range(DC):
                        nc.vector.tensor_scalar_add(qt_t[m], q_ps[m], bq_t[:, m:m + 1])

                # ---- gap fillers while AG(K0) completes ----
                # x1_q.T[m] = Wl[:, D:].T-chunks @ Q.T (+ bl folded in)
                x1q_t = [qfpool.tile([P, R], F32, name=f"x1q{m}") for m in range(DC)]
                x1q_ps = [pspool.tile([P, R], F32, name=f"x1qps{m}", tag="ps")
                          for m in range(DC)]
                for k in range(DC):
                    wl_t = wpool.tile([P, D], F16, name="wl_t", tag="w")
                    nc.sync.dma_start(wl_t, wlt[(DC + k) * P:(DC + k + 1) * P, :])
                    for m in range(DC):
                        nc.tensor.matmul(
                            x1q_ps[m], wl_t[:, m * P:(m + 1) * P], qt_t[k],
                            start=(k == 0), stop=(k == DC - 1))
                for m in range(DC):
                    nc.vector.tensor_scalar_add(x1q_t[m], x1q_ps[m], bl_t[:, m:m + 1])

                # tail HK k-chunks of h-g0's Q part (+ ba folded in)
                hq0_t = [qfpool.tile([P, R], F32, name=f"hq0_{m}") for m in range(DC)]
                hq0_ps = [pspool.tile([P, R], F32, name=f"hq0ps{m}", tag="ps")
                          for m in range(DC)]
                for j in range(HK):
                    k = TDC - HK + j
                    wa_t = wpool.tile([P, D], F16, name="wa_t", tag="w")
                    nc.sync.dma_start(wa_t, wat[k * P:(k + 1) * P, 0:D])
                    for m in range(DC):
                        nc.tensor.matmul(
                            hq0_ps[m], wa_t[:, m * P:(m + 1) * P], qt_t[k - DC],
                            start=(j == 0), stop=(j == HK - 1))
                for m in range(DC):
                    nc.vector.tensor_scalar_add(hq0_t[m], hq0_ps[m], ba_t[:, m:m + 1])

                # ============ scoresT + exp + sums ============
                with tc.tile_pool(name="epool", bufs=1) as epool:
                    exp_t = [epool.tile([P, R], F16, name=f"exp{i}")
                             for i in range(NK)]
                    sums_ps = pspool.tile([1, R], F32, name="sums_ps", tag="ps")

                    def sums_mm(i):
                        nc.tensor.matmul(
                            sums_ps, ones_t, exp_t[i],
                            start=(i == 0), stop=(i == NK - 1),
                            skip_group_check=True)

                    for i, (rr, ss) in enumerate(KT_ORDER):
                        h, sh = (0, ss) if ss < 2 else (1, ss - 2)
                        kl = wpool.tile([P, D], F16, name="kl", tag="w")
                        eng = nc.sync if i % 2 == 0 else nc.scalar
                        eng.dma_start(
                            kl.rearrange("p (c n) -> p c n", c=DC),
                            ag_k[h][rr * D:(rr + 1) * D, sh * P:(sh + 1) * P]
                            .rearrange("(c p) n -> p c n", p=P))
                        sc_ps = pspool.tile([P, R], F32, name="sc_ps", tag="ps")
                        for c in range(DC):
                            nc.tensor.matmul(
                                sc_ps, kl[:, c * P:(c + 1) * P], qt_t[c],
                                start=(c == 0), stop=(c == DC - 1))
                        nc.scalar.activation(
                            exp_t[i], sc_ps, mybir.ActivationFunctionType.Exp,
                            bias=0.0, scale=1.0 / 32.0)
                        if i > 0:
                            sums_mm(i - 1)    # one behind: exp(i-1) surely done
                    sums_mm(NK - 1)

                    # reciprocal + broadcast to all partitions
                    recip_t = cpool.tile([1, R], F32, name="recip_t")
                    nc.vector.reciprocal(recip_t, sums_ps)
                    bc_ps = pspool.tile([P, R], F32, name="bc_ps", tag="ps")
                    nc.tensor.matmul(bc_ps, ones_row, recip_t, start=True, stop=True)
                    bc_t = cpool.tile([P, R], F32, name="bc_t")
                    nc.vector.tensor_copy(bc_t, bc_ps)

                    # ============ V_T = (alpha @ V).T ============
                    vt_ps = [pspool.tile([P, R], F32, name=f"vtps{m}", tag="ps")
                             for m in range(DC)]
                    for i, (rr, ss) in enumerate(KT_ORDER):
                        h, sh = (0, ss) if ss < 2 else (1, ss - 2)
                        vl = vlpool.tile([P, D], F16, name="vl", tag="vl")
                        eng = nc.sync if i % 2 == 0 else nc.scalar
                        eng.dma_start(
                            vl, ag_v[h][rr * HR + sh * P:rr * HR + (sh + 1) * P, :])
                        for m in range(DC):
                            nc.tensor.matmul(
                                vt_ps[m], vl[:, m * P:(m + 1) * P], exp_t[i],
                                start=(i == 0), stop=(i == NK - 1),
                                skip_group_check=True)
                    vt_t = [vtpool.tile([P, R], F16, name=f"vt{m}")
                            for m in range(DC)]
                    for m in range(DC):
                        nc.vector.tensor_mul(vt_t[m], vt_ps[m], bc_t)

                # ============ x1 (V-half), h, GLU ============
                def xk(k):
                    return vt_t[k] if k < DC else qt_t[k - DC]

                with tc.tile_pool(name="fpool", bufs=1) as fpool, \
                     tc.tile_pool(name="wg1pool", bufs=1) as wg1pool:
                    x1_ps = [pspool.tile([P, R], F32, name=f"x1ps{m}", tag="ps")
                             for m in range(DC)]
                    for k in range(DC):
                        wl_t = wpool.tile([P, D], F16, name="wl_t", tag="w")
                        nc.sync.dma_start(wl_t, wlt[k * P:(k + 1) * P, :])
                        for m in range(DC):
                            nc.tensor.matmul(
                                x1_ps[m], wl_t[:, m * P:(m + 1) * P], vt_t[k],
                                start=(k == 0), stop=(k == DC - 1))
                    x1_t = [fpool.tile([P, R], F32, name=f"x1{m}") for m in range(DC)]
                    for m in range(DC):
                        nc.vector.tensor_add(x1_t[m], x1_ps[m], x1q_t[m])

                    # h group 0 (a part): k-chunks [0, TDC-HK), Q-tail was
                    # precomputed into hq0_t
                    a_t = [fpool.tile([P, R], F32, name=f"a{m}") for m in range(DC)]
                    h_ps = [pspool.tile([P, R], F32, name=f"hps0_{m}", tag="ps")
                            for m in range(DC)]
                    for k in range(TDC - HK):
                        wa_t = wpool.tile([P, D], F16, name="wa_t", tag="w")
                        nc.sync.dma_start(wa_t, wat[k * P:(k + 1) * P, 0:D])
                        for m in range(DC):
                            nc.tensor.matmul(
                                h_ps[m], wa_t[:, m * P:(m + 1) * P], xk(k),
                                start=(k == 0), stop=(k == TDC - HK - 1))
                    for m in range(DC):
                        nc.vector.tensor_add(a_t[m], h_ps[m], hq0_t[m])

                    # h group 1 (b part): preload all 16 wa tiles, loop
                    # m-outer so each output column block finishes early and
                    # the GLU/output tail overlaps remaining matmuls.
                    wg1_t = [wg1pool.tile([P, D], F16, name=f"wg1_{k}")
                             for k in range(TDC)]
                    for k in range(TDC):
                        eng = nc.sync if k % 2 == 0 else nc.scalar
                        eng.dma_start(wg1_t[k], wat[k * P:(k + 1) * P, D:TD])
                    for m in range(DC):
                        hg1 = pspool.tile([P, R], F32, name=f"hps1_{m}", tag="ps")
                        for k in range(TDC):
                            nc.tensor.matmul(
                                hg1, wg1_t[k][:, m * P:(m + 1) * P], xk(k),
                                start=(k == 0), stop=(k == TDC - 1))
                        sig = fpool.tile([P, R], F32, name="sig", tag="sig", bufs=2)
                        nc.scalar.activation(
                            sig, hg1, mybir.ActivationFunctionType.Sigmoid,
                            bias=ba_t[:, DC + m:DC + m + 1], scale=1.0)
                        nc.vector.tensor_mul(a_t[m], a_t[m], sig)
                        nc.vector.tensor_mul(a_t[m], x1_t[m], a_t[m])
                        nc.scalar.dma_start(out[m * P:(m + 1) * P, :], a_t[m])

    nc.compile()
    return nc


_NC = None


def _get_nc():
    global _NC
    if _NC is None:
        _NC = build_nc()
    return _NC


def make_in_maps(input_features, Wq, bq, Wk, bk, Wv, bv, Wl, bl, Wa, ba):
    f = np.ascontiguousarray
    x = np.asarray(input_features, dtype=np.float32)
    xt_full = f(x.T.astype(np.float16))                  # [D, N]
    wqt = f(np.asarray(Wq, np.float32).T.astype(np.float16))
    wkt = f(np.asarray(Wk, np.float32).T.astype(np.float16))
    wvt = f(np.asarray(Wv, np.float32).T.astype(np.float16))
    wlt = f(np.asarray(Wl, np.float32).T.astype(np.float16))   # [2D, D]
    wat = f(np.asarray(Wa, np.float32).T.astype(np.float16))   # [2D, 2D]
    bq_r = f(np.asarray(bq, np.float32).reshape(DC, P).T)      # [P, DC]
    bk_r = f(np.asarray(bk, np.float32).reshape(DC, P).T)
    bl_r = f(np.asarray(bl, np.float32).reshape(DC, P).T)
    ba_r = f(np.asarray(ba, np.float32).reshape(TDC, P).T)     # [P, TDC]
    bvb = f(np.broadcast_to(np.asarray(bv, np.float32), (P, D)))
    in_maps = []
    for c in range(NCORES):
        in_maps.append({
            "xt": f(xt_full[:, c * R:(c + 1) * R]),
            "wqt": wqt, "wkt": wkt, "wvt": wvt, "wlt": wlt, "wat": wat,
            "bq": bq_r, "bk": bk_r, "bvb": bvb, "bl": bl_r, "ba": ba_r,
        })
    return in_maps


def run(in_maps, trace=False):
    nc = _get_nc()
    return bass_utils.run_bass_kernel_spmd(
        nc, in_maps, core_ids=list(range(NCORES)), trace=trace)


def kernel(input_features, Wq, bq, Wk, bk, Wv, bv, Wl, bl, Wa, ba):
    in_maps = make_in_maps(input_features, Wq, bq, Wk, bk, Wv, bv, Wl, bl, Wa, ba)
    res = run(in_maps)
    out = np.empty((N, D), dtype=np.float32)
    for c in range(NCORES):
        out[c * R:(c + 1) * R, :] = res.results[c]["out"].T
    return out



# revision 6
# speedup vs baseline: 1.1835x; 1.1835x over previous
"""IntraAttention Trainium2 kernel, 8-core SPMD, fp8 DoubleRow edition.

Reference computation (N=4096 rows, d=1024):
    Q = X @ Wq.T + bq ; K = X @ Wk.T + bk ; V = X @ Wv.T + bv
    alpha = softmax(Q @ K.T / sqrt(d), axis=1)
    V_ = alpha @ V
    x = concat([V_, Q], axis=1)              # [N, 2d]
    x1 = x @ Wl.T + bl                        # [N, d]
    h = x @ Wa.T + ba                         # [N, 2d]
    out = x1 * (h[:, :d] * sigmoid(h[:, d:]))

Sharding: rows of X sharded across 8 cores (512 rows each). Q local;
K and V shards all-gathered in fp8 (K as two kt-half chunks in a
swizzled [p, (kt, c, n)] layout so per-tile loads are contiguous, V as
two row-half chunks). All error-tolerant matmuls (K/V projections,
scores, alpha@V) run fp8e4 DoubleRow (256-deep contraction per MM);
the Q projection and the concat->linear->GLU chain stay fp16 for
accuracy. While the gathers fly, the PE precomputes the full Q-half of
x1 and both halves of h (192 fp16 MMs of filler).
"""

import numpy as np
import ml_dtypes

import concourse.bass as bass
import concourse.bacc as bacc
import concourse.tile as tile
import concourse.bass_utils as bass_utils
from concourse import mybir

P = 128            # partitions
D = 1024           # model dim
N = 4096           # rows
NCORES = 8
R = N // NCORES    # rows per core = 512
DC = D // P        # d chunks = 8
TD = 2 * D         # 2048
TDC = TD // P      # 16
KT = 4             # local key tiles (of 128 keys) per rank
NPAIR = 16         # global key-tile pairs (256 keys each)

F32 = mybir.dt.float32
F16 = mybir.dt.float16
F8 = mybir.dt.float8e4
DR = mybir.MatmulPerfMode.DoubleRow

RG = [list(range(NCORES))]


def build_nc():
    nc = bacc.Bacc(
        "TRN2",
        target_bir_lowering=False,
        debug=False,
        num_devices=NCORES,
    )

    # ---- per-core I/O ----
    xt = nc.dram_tensor("xt", [D, R], F16, kind="ExternalInput")      # X_c.T
    xt8 = nc.dram_tensor("xt8", [D, R], F8, kind="ExternalInput")     # X_c.T
    wqt = nc.dram_tensor("wqt", [D, D], F16, kind="ExternalInput")    # Wq.T
    wk8 = nc.dram_tensor("wk8", [D, D], F8, kind="ExternalInput")     # Wk.T
    wv8 = nc.dram_tensor("wv8", [D, D], F8, kind="ExternalInput")     # Wv.T
    wlt = nc.dram_tensor("wlt", [TD, D], F16, kind="ExternalInput")   # Wl.T
    wat = nc.dram_tensor("wat", [TD, TD], F16, kind="ExternalInput")  # Wa.T
    bq = nc.dram_tensor("bq", [P, DC], F32, kind="ExternalInput")
    bk = nc.dram_tensor("bk", [P, DC], F32, kind="ExternalInput")
    bvb = nc.dram_tensor("bvb", [P, D], F32, kind="ExternalInput")    # bv bcast
    bl = nc.dram_tensor("bl", [P, DC], F32, kind="ExternalInput")
    ba = nc.dram_tensor("ba", [P, TDC], F32, kind="ExternalInput")
    out = nc.dram_tensor("out", [D, R], F32, kind="ExternalOutput")   # out_c.T

    # ---- collective buffers (fp8) ----
    # K.T swizzled per half h: row p holds [s(kt within half), c, n] so a
    # key-tile-pair load is one contiguous 2KB-per-partition DMA.
    kst_d = [nc.dram_tensor(f"kst_d{h}", [P, 2 * DC * P], F8) for h in range(2)]
    ag_k = [nc.dram_tensor(f"ag_k{h}", [NCORES * P, 2 * DC * P], F8,
                           addr_space="Shared") for h in range(2)]
    # V rows, plain layout, split in two row-halves per rank.
    vc_d = [nc.dram_tensor(f"vc_d{h}", [R // 2, D], F8) for h in range(2)]
    ag_v = [nc.dram_tensor(f"ag_v{h}", [NCORES * (R // 2), D], F8,
                           addr_space="Shared") for h in range(2)]

    with tile.TileContext(nc) as tc:
        with (
            tc.tile_pool(name="cpool", bufs=1) as cpool,
            tc.tile_pool(name="wpool", bufs=8) as wpool,
            tc.tile_pool(name="vlpool", bufs=4) as vlpool,
            tc.tile_pool(name="pspool", bufs=8, space="PSUM") as pspool,
        ):
            # constants on the scalar DMA queue
            bq_t = cpool.tile([P, DC], F32, name="bq_t")
            bk_t = cpool.tile([P, DC], F32, name="bk_t")
            bl_t = cpool.tile([P, DC], F32, name="bl_t")
            ba_t = cpool.tile([P, TDC], F32, name="ba_t")
            bvb_t = cpool.tile([P, D], F32, name="bvb_t")
            nc.scalar.dma_start(bq_t, bq[:, :])
            nc.scalar.dma_start(bk_t, bk[:, :])
            nc.scalar.dma_start(bl_t, bl[:, :])
            nc.scalar.dma_start(ba_t, ba[:, :])
            nc.scalar.dma_start(bvb_t, bvb[:, :])
            ones8_t = cpool.tile([P, 1], F8, name="ones8_t")
            nc.vector.memset(ones8_t, 1.0)
            ones_row = cpool.tile([1, P], F32, name="ones_row")
            nc.vector.memset(ones_row, 1.0)

            with tc.tile_pool(name="qpool", bufs=1) as qpool, \
                 tc.tile_pool(name="vtpool", bufs=1) as vtpool, \
                 tc.tile_pool(name="qfpool", bufs=1) as qfpool:

                # ============ K/V/Q projections + allgathers ============
                with tc.tile_pool(name="xpool", bufs=1) as xpool, \
                     tc.tile_pool(name="kvwpool", bufs=1) as kvwpool, \
                     tc.tile_pool(name="stpool", bufs=4) as stpool, \
                     tc.tile_pool(name="kstpool", bufs=1) as kstpool:

                    # --- K_c.T = Wk @ X_c.T + bk  (fp8 DoubleRow) ---
                    xt8_t = [xpool.tile([P, 2, R], F8, name=f"xt8_{j}")
                             for j in range(4)]
                    wk8_t = [kvwpool.tile([P, 2, D], F8, name=f"wk8_{j}")
                             for j in range(4)]
                    for j in range(4):
                        nc.sync.dma_start(
                            wk8_t[j],
                            wk8[2 * j * P:(2 * j + 2) * P, :]
                            .rearrange("(s p) c -> p s c", p=P))
                        nc.sync.dma_start(
                            xt8_t[j],
                            xt8[2 * j * P:(2 * j + 2) * P, :]
                            .rearrange("(s p) n -> p s n", p=P))
                    # swizzled K staging: [P, s, c*128+n] per half
                    kst_t = [kstpool.tile([P, 2, DC * P], F8, name=f"kst_{h}")
                             for h in range(2)]
                    for m in range(DC):
                        kt_ps = pspool.tile([P, R], F32, name="kt_ps", tag="ps")
                        for j in range(4):
                            nc.tensor.matmul(
                                kt_ps, wk8_t[j][:, :, m * P:(m + 1) * P],
                                xt8_t[j], start=(j == 0), stop=(j == 3),
                                perf_mode=DR)
                        for h in range(2):
                            nc.vector.tensor_scalar_add(
                                kst_t[h][:, :, m * P:(m + 1) * P],
                                kt_ps[:, h * 2 * P:(h + 1) * 2 * P]
                                .rearrange("p (s n) -> p s n", s=2),
                                bk_t[:, m:m + 1])
                    for h in range(2):
                        nc.scalar.dma_start(
                            kst_d[h][:, :],
                            kst_t[h].rearrange("p s n -> p (s n)"))
                    for h in range(2):
                        nc.gpsimd.collective_compute(
                            "AllGather", mybir.AluOpType.bypass, replica_groups=RG,
                            ins=[kst_d[h].ap().opt()], outs=[ag_k[h].ap().opt()])

                    # --- V_c = X_c @ Wv.T + bv  (fp8 DoubleRow) ---
                    wv8_t = [kvwpool.tile([P, 2, D], F8, name=f"wv8_{j}")
                             for j in range(4)]
                    for j in range(4):
                        nc.sync.dma_start(
                            wv8_t[j],
                            wv8[2 * j * P:(2 * j + 2) * P, :]
                            .rearrange("(s p) c -> p s c", p=P))
                    for o in range(8):
                        rt, db = o // 2, o % 2
                        v_ps = pspool.tile([P, R], F32, name="v_ps", tag="ps")
                        for j in range(4):
                            nc.tensor.matmul(
                                v_ps, xt8_t[j][:, :, rt * P:(rt + 1) * P],
                                wv8_t[j][:, :, db * 512:(db + 1) * 512],
                                start=(j == 0), stop=(j == 3), perf_mode=DR)
                        st = stpool.tile([P, 512], F8, name="st_v", tag="st")
                        nc.vector.tensor_add(
                            st, v_ps, bvb_t[:, db * 512:(db + 1) * 512])
                        nc.scalar.dma_start(
                            vc_d[rt // 2][(rt % 2) * P:(rt % 2 + 1) * P,
                                          db * 512:(db + 1) * 512], st)
                    for h in range(2):
                        nc.gpsimd.collective_compute(
                            "AllGather", mybir.AluOpType.bypass, replica_groups=RG,
                            ins=[vc_d[h].ap().opt()], outs=[ag_v[h].ap().opt()])

                # --- Q_c.T = Wq @ X_c.T + bq  (fp16) ---
                qt_t = [qpool.tile([P, R], F16, name=f"qt{m}") for m in range(DC)]
                qt8_t = qpool.tile([P, DC * R], F8, name="qt8")
                with tc.tile_pool(name="xqpool", bufs=1) as xqpool, \
                     tc.tile_pool(name="wqpool", bufs=1) as wqpool:
                    xt_t = [xqpool.tile([P, R], F16, name=f"xt{k}")
                            for k in range(DC)]
                    wq_t = [wqpool.tile([P, D], F16, name=f"wq_{k}")
                            for k in range(DC)]
                    for k in range(DC):
                        nc.sync.dma_start(xt_t[k], xt[k * P:(k + 1) * P, :])
                        nc.sync.dma_start(wq_t[k], wqt[k * P:(k + 1) * P, :])
                    for m in range(DC):
                        q_ps = pspool.tile([P, R], F32, name="q_ps", tag="ps")
                        for k in range(DC):
                            nc.tensor.matmul(
                                q_ps, wq_t[k][:, m * P:(m + 1) * P], xt_t[k],
                                start=(k == 0), stop=(k == DC - 1))
                        nc.vector.tensor_scalar_add(qt_t[m], q_ps, bq_t[:, m:m + 1])
                        nc.vector.tensor_scalar_add(
                            qt8_t[:, m * R:(m + 1) * R], q_ps, bq_t[:, m:m + 1])

                # ---- Q-half fillers while the allgathers fly ----
                # x1_q.T[m], h_a_q.T[m], h_b_q.T[m]: Wl/Wa rows D..2D against Q
                x1q_t = [qfpool.tile([P, R], F32, name=f"x1q{m}") for m in range(DC)]
                hqa_t = [qfpool.tile([P, R], F32, name=f"hqa{m}") for m in range(DC)]
                hqb_t = [qfpool.tile([P, R], F32, name=f"hqb{m}") for m in range(DC)]
                with tc.tile_pool(name="fwpool", bufs=1) as fwpool:
                    wlq_t = [fwpool.tile([P, D], F16, name=f"wlq{k}")
                             for k in range(DC)]
                    waqa_t = [fwpool.tile([P, D], F16, name=f"waqa{k}")
                              for k in range(DC)]
                    waqb_t = [fwpool.tile([P, D], F16, name=f"waqb{k}")
                              for k in range(DC)]
                    for k in range(DC):
                        nc.sync.dma_start(
                            wlq_t[k], wlt[(DC + k) * P:(DC + k + 1) * P, :])
                    for k in range(DC):
                        eng = nc.sync if k % 2 == 0 else nc.scalar
                        eng.dma_start(
                            waqa_t[k], wat[(DC + k) * P:(DC + k + 1) * P, 0:D])
                        eng.dma_start(
                            waqb_t[k], wat[(DC + k) * P:(DC + k + 1) * P, D:TD])
                    for m in range(DC):
                        ps = pspool.tile([P, R], F32, name="x1q_ps", tag="ps")
                        for k in range(DC):
                            nc.tensor.matmul(
                                ps, wlq_t[k][:, m * P:(m + 1) * P], qt_t[k],
                                start=(k == 0), stop=(k == DC - 1))
                        nc.vector.tensor_scalar_add(x1q_t[m], ps, bl_t[:, m:m + 1])
                    for m in range(DC):
                        ps = pspool.tile([P, R], F32, name="hqa_ps", tag="ps")
                        for k in range(DC):
                            nc.tensor.matmul(
                                ps, waqa_t[k][:, m * P:(m + 1) * P], qt_t[k],
                                start=(k == 0), stop=(k == DC - 1))
                        nc.vector.tensor_scalar_add(hqa_t[m], ps, ba_t[:, m:m + 1])
                    for m in range(DC):
                        ps = pspool.tile([P, R], F32, name="hqb_ps", tag="ps")
                        for k in range(DC):
                            nc.tensor.matmul(
                                ps, waqb_t[k][:, m * P:(m + 1) * P], qt_t[k],
                                start=(k == 0), stop=(k == DC - 1))
                        nc.vector.tensor_scalar_add(
                            hqb_t[m], ps, ba_t[:, DC + m:DC + m + 1])

                # ============ scoresT + exp + sums (fp8 DR) ============
                # pair pi: kp = pi // 8 (kt-half), rr = pi % 8 (source rank);
                # covers global keys rr*512 + kp*256 + s*128 + p.
                with tc.tile_pool(name="epool", bufs=1) as epool, \
                     tc.tile_pool(name="kpool", bufs=4) as kpool:
                    exp_p = [epool.tile([P, 2 * R], F8, name=f"exp{pi}")
                             for pi in range(NPAIR)]
                    sums_ps = pspool.tile([1, R], F32, name="sums_ps", tag="ps")
                    n_exp = 0

                    def sums_mm(pj, sj, idx):
                        nc.tensor.matmul(
                            sums_ps, ones8_t, exp_p[pj][:, sj * R:(sj + 1) * R],
                            start=(idx == 0), stop=(idx == 2 * NPAIR - 1),
                            skip_group_check=True)

                    for pi in range(NPAIR):
                        kp, rr = pi // 8, pi % 8
                        kls = [kpool.tile([P, DC * P], F8, name="kls", tag="kl")
                               for _ in range(2)]
                        eng = nc.sync if pi % 2 == 0 else nc.scalar
                        for s in range(2):
                            eng.dma_start(
                                kls[s],
                                ag_k[kp][rr * P:(rr + 1) * P,
                                         s * DC * P:(s + 1) * DC * P])
                        for s in range(2):
                            sc_ps = pspool.tile([P, R], F32, name="sc_ps", tag="ps")
                            for cp in range(4):
                                nc.tensor.matmul(
                                    sc_ps,
                                    kls[s][:, 2 * cp * P:(2 * cp + 2) * P]
                                    .rearrange("p (c n) -> p c n", c=2),
                                    qt8_t[:, 2 * cp * R:(2 * cp + 2) * R]
                                    .rearrange("p (c n) -> p c n", c=2),
                                    start=(cp == 0), stop=(cp == 3),
                                    perf_mode=DR)
                            nc.scalar.activation(
                                exp_p[pi][:, s * R:(s + 1) * R], sc_ps,
                                mybir.ActivationFunctionType.Exp,
                                bias=0.0, scale=1.0 / 32.0)
                            if n_exp > 0:
                                pj = (n_exp - 1) // 2
                                sums_mm(pj, (n_exp - 1) % 2, n_exp - 1)
                            n_exp += 1
                    sums_mm(NPAIR - 1, 1, 2 * NPAIR - 1)

                    # reciprocal + broadcast to all partitions
                    recip_t = cpool.tile([1, R], F32, name="recip_t")
                    nc.vector.reciprocal(recip_t, sums_ps)
                    bc_ps = pspool.tile([P, R], F32, name="bc_ps", tag="ps")
                    nc.tensor.matmul(bc_ps, ones_row, recip_t, start=True, stop=True)
                    bc_t = cpool.tile([P, R], F32, name="bc_t")
                    nc.vector.tensor_copy(bc_t, bc_ps)

                    # ============ V_T = (alpha @ V).T  (fp8 DR) ============
                    vt_ps = [pspool.tile([P, R], F32, name=f"vtps{m}", tag="ps")
                             for m in range(DC)]
                    for pi in range(NPAIR):
                        kp, rr = pi // 8, pi % 8
                        vl2 = vlpool.tile([P, 2, D], F8, name="vl2", tag="vl")
                        eng = nc.sync if pi % 2 == 0 else nc.scalar
                        base = rr * 256
                        eng.dma_start(
                            vl2,
                            ag_v[kp][base:base + 256, :]
                            .rearrange("(s p) d -> p s d", p=P))
                        for m in range(DC):
                            nc.tensor.matmul(
                                vt_ps[m], vl2[:, :, m * P:(m + 1) * P],
                                exp_p[pi].rearrange("p (s n) -> p s n", s=2),
                                start=(pi == 0), stop=(pi == NPAIR - 1),
                                perf_mode=DR, skip_group_check=True)
                    vt_t = [vtpool.tile([P, R], F16, name=f"vt{m}")
                            for m in range(DC)]
                    for m in range(DC):
                        nc.vector.tensor_mul(vt_t[m], vt_ps[m], bc_t)

                # ============ x1 (V-half), h (V-half), GLU ============
                with tc.tile_pool(name="fpool", bufs=1) as fpool, \
                     tc.tile_pool(name="vwpool", bufs=1) as vwpool, \
                     tc.tile_pool(name="opool", bufs=2) as opool:
                    wlv_t = [vwpool.tile([P, D], F16, name=f"wlv{k}")
                             for k in range(DC)]
                    wava_t = [vwpool.tile([P, D], F16, name=f"wava{k}")
                              for k in range(DC)]
                    wavb_t = [vwpool.tile([P, D], F16, name=f"wavb{k}")
                              for k in range(DC)]
                    for k in range(DC):
                        eng = nc.sync if k % 2 == 0 else nc.scalar
                        eng.dma_start(wlv_t[k], wlt[k * P:(k + 1) * P, :])
                        eng.dma_start(wava_t[k], wat[k * P:(k + 1) * P, 0:D])
                    for k in range(DC):
                        eng = nc.sync if k % 2 == 0 else nc.scalar
                        eng.dma_start(wavb_t[k], wat[k * P:(k + 1) * P, D:TD])

                    x1_t = [fpool.tile([P, R], F32, name=f"x1{m}") for m in range(DC)]
                    for m in range(DC):
                        ps = pspool.tile([P, R], F32, name="x1_ps", tag="ps")
                        for k in range(DC):
                            nc.tensor.matmul(
                                ps, wlv_t[k][:, m * P:(m + 1) * P], vt_t[k],
                                start=(k == 0), stop=(k == DC - 1))
                        nc.vector.tensor_add(x1_t[m], ps, x1q_t[m])

                    a_t = [fpool.tile([P, R], F32, name=f"a{m}") for m in range(DC)]
                    for m in range(DC):
                        ps = pspool.tile([P, R], F32, name="hva_ps", tag="ps")
                        for k in range(DC):
                            nc.tensor.matmul(
                                ps, wava_t[k][:, m * P:(m + 1) * P], vt_t[k],
                                start=(k == 0), stop=(k == DC - 1))
                        nc.vector.tensor_add(a_t[m], ps, hqa_t[m])

                    # b half m-outer so GLU/output tail overlaps remaining MMs
                    for m in range(DC):
                        ps = pspool.tile([P, R], F32, name="hvb_ps", tag="ps")
                        for k in range(DC):
                            nc.tensor.matmul(
                                ps, wavb_t[k][:, m * P:(m + 1) * P], vt_t[k],
                                start=(k == 0), stop=(k == DC - 1))
                        b_t = opool.tile([P, R], F32, name="b_t", tag="bt")
                        nc.vector.tensor_add(b_t, ps, hqb_t[m])
                        sig = opool.tile([P, R], F32, name="sig", tag="sig")
                        nc.scalar.activation(
                            sig, b_t, mybir.ActivationFunctionType.Sigmoid,
                            bias=0.0, scale=1.0)
                        nc.vector.tensor_mul(a_t[m], a_t[m], sig)
                        nc.vector.tensor_mul(a_t[m], x1_t[m], a_t[m])
                        nc.scalar.dma_start(out[m * P:(m + 1) * P, :], a_t[m])

    nc.compile()
    return nc


_NC = None


def _get_nc():
    global _NC
    if _NC is None:
        _NC = build_nc()
    return _NC


def make_in_maps(input_features, Wq, bq, Wk, bk, Wv, bv, Wl, bl, Wa, ba):
    f = np.ascontiguousarray
    FP8 = ml_dtypes.float8_e4m3
    x = np.asarray(input_features, dtype=np.float32)
    xt_full = f(x.T.astype(np.float16))                  # [D, N]
    xt8_full = f(x.T.astype(FP8))
    wqt = f(np.asarray(Wq, np.float32).T.astype(np.float16))
    wk8 = f(np.asarray(Wk, np.float32).T.astype(FP8))
    wv8 = f(np.asarray(Wv, np.float32).T.astype(FP8))
    wlt = f(np.asarray(Wl, np.float32).T.astype(np.float16))   # [2D, D]
    wat = f(np.asarray(Wa, np.float32).T.astype(np.float16))   # [2D, 2D]
    bq_r = f(np.asarray(bq, np.float32).reshape(DC, P).T)      # [P, DC]
    bk_r = f(np.asarray(bk, np.float32).reshape(DC, P).T)
    bl_r = f(np.asarray(bl, np.float32).reshape(DC, P).T)
    ba_r = f(np.asarray(ba, np.float32).reshape(TDC, P).T)     # [P, TDC]
    bvb = f(np.broadcast_to(np.asarray(bv, np.float32), (P, D)))
    in_maps = []
    for c in range(NCORES):
        in_maps.append({
            "xt": f(xt_full[:, c * R:(c + 1) * R]),
            "xt8": f(xt8_full[:, c * R:(c + 1) * R]),
            "wqt": wqt, "wk8": wk8, "wv8": wv8, "wlt": wlt, "wat": wat,
            "bq": bq_r, "bk": bk_r, "bvb": bvb, "bl": bl_r, "ba": ba_r,
        })
    return in_maps


def run(in_maps, trace=False):
    nc = _get_nc()
    return bass_utils.run_bass_kernel_spmd(
        nc, in_maps, core_ids=list(range(NCORES)), trace=trace)


def kernel(input_features, Wq, bq, Wk, bk, Wv, bv, Wl, bl, Wa, ba):
    in_maps = make_in_maps(input_features, Wq, bq, Wk, bk, Wv, bv, Wl, bl, Wa, ba)
    res = run(in_maps)
    out = np.empty((N, D), dtype=np.float32)
    for c in range(NCORES):
        out[c * R:(c + 1) * R, :] = res.results[c]["out"].T
    return out
